# revision 1
# baseline (speedup 1.0000x reference)
"""LRU (diagonal complex linear recurrence) Trainium2 Bass kernel, v9.

Math (per batch b, channel h, time t = 0..L-1):
    u_t   = delta * (x_t @ B_real + i * x_t @ B_img)
    h_t   = lam * h_{t-1} + u_t,   h_{-1} = h0,  lam = r e^{i theta}
    out_t = Re(h_t)

Structure — time-decimated (m=2) polar scan with the pair-combine
folded into the GEMM:
  E_j := h_{2j} obeys E_j = lam^2 E_{j-1} + u~_j with
  u~_j = lam*u_{2j-1} + u_{2j} = x_{2j-1} @ (lam*Bd) + x_{2j} @ Bd.
  So a GEMM over pairs (x_{2j-1}, x_{2j}) with weight sets
  {btr2,bti2} = lam*(btr+i*bti) and {btr,bti} yields u~ directly at
  half resolution. Polar trick on lam^2 = r^2 e^{i*2theta}:
  E_j = e^{i*2theta(j+1)} G_j,  G_j = r^2 G_{j-1} + e^{-i*2theta(j+1)} u~_j,
  G_{-1} = h0 / lam  (host-computed, fp32).
  Even outputs: out_{2j}  = C.Gr - S.Gi           (C,S = cos/sin 2theta(j+1))
  Odd  outputs: out_{2j+1} = P1.Gr - P2.Gi + Re(u_{2j+1})
                (P1,P2 = r*cos/sin(theta(2j+3)); Re(u_odd) from a third
                 partial GEMM x_{2j+1} @ btr).
  Scans run on DVE at half resolution (scan is the only serial resource).
  ALL elementwise ops run on DVE in fp16 2x mode: concurrent GpSimd
  streaming degrades DVE tensor-tensor throughput ~4x (SBUF port
  contention, measured), so GpSimd is left idle. PSUM->SBUF staging on
  ScalarE (no DVE interference). x is pre-transposed host-side so device
  loads are plain contiguous DMAs. Even/odd outputs go to separate fp16
  DRAM tensors (all stores contiguous); host interleaves + casts.

Sharding: batch-parallel over 8 cores (2 batch elements each), SPMD.
"""

from contextlib import ExitStack

import numpy as np

import concourse.bass as bass
import concourse.tile as tile
from concourse import bacc, mybir

B, L, F, H = 16, 4096, 512, 512
N_CORES = 8
B_LOC = B // N_CORES
HG = H // 128
FG = F // 128
J = L // 2            # half-res length
JO = J + 1            # odd x stream incl. left pad
JOP = 2064            # padded to multiple of 16 for xbar transpose
W = 1024              # elementwise tile width (j-cols)
NW = J // W           # 2 elementwise tiles per (hg, b)
PW = 512              # PSUM gemm tile width (j-cols)
FP32 = mybir.dt.float32
F16 = mybir.dt.float16
A = mybir.AluOpType


def build_program():
    nc = bacc.Bacc("TRN2", target_bir_lowering=False, debug=False,
                   enable_asserts=False, num_devices=1)

    xe_d = nc.dram_tensor("xe", [B_LOC, F, J], F16, kind="ExternalInput").ap()
    xo_d = nc.dram_tensor("xo", [B_LOC, F, JOP], F16, kind="ExternalInput").ap()
    br_d = nc.dram_tensor("btr", [F, H], F16, kind="ExternalInput").ap()
    bi_d = nc.dram_tensor("bti", [F, H], F16, kind="ExternalInput").ap()
    br2_d = nc.dram_tensor("btr2", [F, H], F16, kind="ExternalInput").ap()
    bi2_d = nc.dram_tensor("bti2", [F, H], F16, kind="ExternalInput").ap()
    r2_d = nc.dram_tensor("r2vec", [H], FP32, kind="ExternalInput").ap()
    c_d = nc.dram_tensor("ctab", [H, J], F16, kind="ExternalInput").ap()
    s_d = nc.dram_tensor("stab", [H, J], F16, kind="ExternalInput").ap()
    p1_d = nc.dram_tensor("p1tab", [H, J], F16, kind="ExternalInput").ap()
    p2_d = nc.dram_tensor("p2tab", [H, J], F16, kind="ExternalInput").ap()
    gr0_d = nc.dram_tensor("ginr", [H], FP32, kind="ExternalInput").ap()
    gi0_d = nc.dram_tensor("gini", [H], FP32, kind="ExternalInput").ap()
    oute_d = nc.dram_tensor("oute", [B_LOC, H, J], F16, kind="ExternalOutput").ap()
    outo_d = nc.dram_tensor("outo", [B_LOC, H, J], F16, kind="ExternalOutput").ap()

    with tile.TileContext(nc) as tc, ExitStack() as ctx:
        singles = ctx.enter_context(tc.tile_pool(name="singles", bufs=1))
        xt_pool = ctx.enter_context(tc.tile_pool(name="xt", bufs=1))
        tab_pool = ctx.enter_context(tc.tile_pool(name="tabs", bufs=2))
        u_pool = ctx.enter_context(tc.tile_pool(name="u", bufs=2))
        work = ctx.enter_context(tc.tile_pool(name="work", bufs=2))
        opool = ctx.enter_context(tc.tile_pool(name="opool", bufs=3))
        ps_ab = ctx.enter_context(tc.tile_pool(name="ps_ab", bufs=1, space="PSUM"))
        ps_c = ctx.enter_context(tc.tile_pool(name="ps_c", bufs=4, space="PSUM"))
        ps_d = ctx.enter_context(tc.tile_pool(name="ps_d", bufs=2, space="PSUM"))

        # weights: [128 f-part, FG, H]
        wr = singles.tile([128, FG, H], F16)
        wi = singles.tile([128, FG, H], F16)
        wr2 = singles.tile([128, FG, H], F16)
        wi2 = singles.tile([128, FG, H], F16)
        # wr2/wr first: the first PSUM tile's matmuls need them
        nc.sync.dma_start(out=wr2, in_=br2_d.rearrange("(fg p) h -> p fg h", p=128))
        nc.sync.dma_start(out=wr, in_=br_d.rearrange("(fg p) h -> p fg h", p=128))
        nc.sync.dma_start(out=wi2, in_=bi2_d.rearrange("(fg p) h -> p fg h", p=128))
        nc.sync.dma_start(out=wi, in_=bi_d.rearrange("(fg p) h -> p fg h", p=128))

        r2_s = singles.tile([128, HG], FP32)
        gr0_s = singles.tile([128, HG], FP32)
        gi0_s = singles.tile([128, HG], FP32)
        nc.sync.dma_start(out=r2_s, in_=r2_d.rearrange("(hg p) -> p hg", p=128))
        nc.sync.dma_start(out=gr0_s, in_=gr0_d.rearrange("(hg p) -> p hg", p=128))
        nc.sync.dma_start(out=gi0_s, in_=gi0_d.rearrange("(hg p) -> p hg", p=128))
        from concourse.masks import make_identity
        ident = singles.tile([128, 128], F16)
        make_identity(nc, ident)
        nident = singles.tile([128, 128], F16)
        nc.vector.tensor_scalar(nident, ident, -1.0, None, op0=A.mult)

        ones = singles.tile([128, W], FP32)
        nc.vector.memset(ones, 1.0)
        r2bc = singles.tile([128, HG, W], FP32)
        for hg in range(HG):
            nc.vector.tensor_scalar(r2bc[:, hg, :], ones, r2_s[:, hg:hg + 1],
                                    None, op0=A.mult)

        # x streams pre-transposed host-side: plain contiguous loads,
        # quarter-split (separate tiles) so the first GEMM waits for only
        # 1/8 of the x transfer. xte piece q = j-cols [512q, 512q+512);
        # xto piece q = [512q, 512q+528) (16-col overlap covers the +1
        # shifted read of the odd stream). hg=0 tables are issued between
        # the b0 quarters so they don't delay the first GEMM's data.
        xte = [[[xt_pool.tile([128, 512], F16, tag=f"xte{b}_{fg}_{p}",
                              name=f"xte{b}_{fg}_{p}") for p in range(4)]
                for fg in range(FG)] for b in range(B_LOC)]
        xto = [[[xt_pool.tile([128, 528], F16, tag=f"xto{b}_{fg}_{p}",
                              name=f"xto{b}_{fg}_{p}") for p in range(4)]
                for fg in range(FG)] for b in range(B_LOC)]

        def load_quarter(b, q):
            for fg in range(FG):
                fsl = slice(fg * 128, (fg + 1) * 128)
                nc.sync.dma_start(out=xte[b][fg][q],
                                  in_=xe_d[b, fsl, 512 * q:512 * q + 512])
                nc.sync.dma_start(out=xto[b][fg][q],
                                  in_=xo_d[b, fsl, 512 * q:512 * q + 528])

        load_quarter(0, 0)
        load_quarter(0, 1)
        tabs0 = []
        for nm, tsrc in (("ct", c_d), ("st", s_d), ("p1t", p1_d), ("p2t", p2_d)):
            t = tab_pool.tile([128, J], F16, tag=nm, name=f"{nm}0")
            nc.sync.dma_start(out=t, in_=tsrc[0:128, :])
            tabs0.append(t)
        load_quarter(0, 2)
        load_quarter(0, 3)
        for q in range(4):
            load_quarter(1, q)

        pending = []

        def flush_one(p):
            # PE adds (identity matmuls into held PSUM) + ScalarE stores +
            # output DMAs for a supertile finished two iterations ago
            for pc, ssl in p["pcs"]:
                nc.tensor.matmul(pc, ident, p["o3"][:, ssl],
                                 start=False, stop=False)
                nc.tensor.matmul(pc, nident, p["o4"][:, ssl],
                                 start=False, stop=True)
                nc.scalar.copy(out=p["res_o"][:, ssl], in_=pc)
                pd = ps_d.tile([128, PW], FP32, tag="pd")
                nc.tensor.matmul(pd, ident, p["o1"][:, ssl],
                                 start=True, stop=False)
                nc.tensor.matmul(pd, nident, p["o2"][:, ssl],
                                 start=False, stop=True)
                nc.scalar.copy(out=p["res_e"][:, ssl], in_=pd)
            nc.sync.dma_start(out=oute_d[p["b"], p["hsl"], p["jsl"]],
                               in_=p["res_e"][:, :p["w"]])
            nc.sync.dma_start(out=outo_d[p["b"], p["hsl"], p["jsl"]],
                               in_=p["res_o"][:, :p["w"]])

        for hg in range(HG):
            hsl = slice(hg * 128, (hg + 1) * 128)
            if hg == 0:
                ct, st, p1t, p2t = tabs0
            else:
                ct = tab_pool.tile([128, J], F16, tag="ct")
                st = tab_pool.tile([128, J], F16, tag="st")
                p1t = tab_pool.tile([128, J], F16, tag="p1t")
                p2t = tab_pool.tile([128, J], F16, tag="p2t")
                nc.sync.dma_start(out=ct, in_=c_d[hsl, :])
                nc.sync.dma_start(out=st, in_=s_d[hsl, :])
                nc.sync.dma_start(out=p1t, in_=p1_d[hsl, :])
                nc.sync.dma_start(out=p2t, in_=p2_d[hsl, :])

            for b in range(B_LOC):
                gprev = None
                # first (hg0,b0) stream split [512,512,1024] so elementwise
                # starts after only one x quarter has landed
                if hg == 0 and b == 0:
                    jparts = [(0, 512), (512, 512), (1024, 512), (1536, 512)]
                elif hg == 0 and b == 1:
                    jparts = [(0, 512), (512, 512), (1024, 512), (1536, 512)]
                else:
                    jparts = [(0, 1024), (1024, 1024)]
                for j0, w in jparts:
                    if len(pending) >= 2:
                        flush_one(pending.pop(0))
                    ur_sb = u_pool.tile([128, W], F16, tag="ur_sb")
                    ui_sb = u_pool.tile([128, W], F16, tag="ui_sb")
                    pcs = []
                    for ps in range(w // PW):
                        p0 = j0 + ps * PW
                        pq = p0 // 512                # x quarter piece
                        osl = slice(0, PW)
                        osl1 = slice(1, PW + 1)
                        ssl = slice(ps * PW, (ps + 1) * PW)
                        pa = ps_ab.tile([128, PW], FP32, tag="pa")
                        pb = ps_ab.tile([128, PW], FP32, tag="pb")
                        pc = ps_c.tile([128, PW], FP32, tag="pc")
                        for fg in range(FG):
                            nc.tensor.matmul(pa, wr2[:, fg, hsl],
                                             xto[b][fg][pq][:, osl],
                                             start=(fg == 0), stop=False)
                        for fg in range(FG):
                            nc.tensor.matmul(pa, wr[:, fg, hsl],
                                             xte[b][fg][pq][:, osl],
                                             start=False, stop=(fg == FG - 1))
                        for fg in range(FG):
                            nc.tensor.matmul(pb, wi2[:, fg, hsl],
                                             xto[b][fg][pq][:, osl],
                                             start=(fg == 0), stop=False)
                        for fg in range(FG):
                            nc.tensor.matmul(pb, wi[:, fg, hsl],
                                             xte[b][fg][pq][:, osl],
                                             start=False, stop=(fg == FG - 1))
                        for fg in range(FG):
                            nc.tensor.matmul(pc, wr[:, fg, hsl],
                                             xto[b][fg][pq][:, osl1],
                                             start=(fg == 0), stop=False)
                        nc.scalar.copy(out=ur_sb[:, ssl], in_=pa)
                        nc.scalar.copy(out=ui_sb[:, ssl], in_=pb)
                        pcs.append((pc, ssl))

                    jsl = slice(j0, j0 + w)
                    cw = ct[:, jsl]
                    sw = st[:, jsl]
                    # input rotation: v = e^{-i*2theta(j+1)} u~
                    t1 = work.tile([128, W], F16, tag="t1")
                    t2 = work.tile([128, W], F16, tag="t2")
                    t3 = work.tile([128, W], F16, tag="t3")
                    t4 = work.tile([128, W], F16, tag="t4")
                    vr = work.tile([128, W], F16, tag="vr")
                    vi = work.tile([128, W], F16, tag="vi")
                    nc.vector.tensor_mul(t1[:, :w], cw, ur_sb[:, :w])
                    nc.vector.tensor_mul(t2[:, :w], sw, ui_sb[:, :w])
                    nc.vector.tensor_add(vr[:, :w], t1[:, :w], t2[:, :w])
                    nc.vector.tensor_mul(t3[:, :w], cw, ui_sb[:, :w])
                    nc.vector.tensor_mul(t4[:, :w], sw, ur_sb[:, :w])
                    nc.vector.tensor_sub(vi[:, :w], t3[:, :w], t4[:, :w])

                    gr = work.tile([128, W], F16, tag="gr")
                    gi = work.tile([128, W], F16, tag="gi")
                    if j0 == 0:
                        init_r = gr0_s[:, hg:hg + 1]
                        init_i = gi0_s[:, hg:hg + 1]
                    else:
                        gr_p, gi_p, wp = gprev
                        init_r = gr_p[:, wp - 1:wp]
                        init_i = gi_p[:, wp - 1:wp]
                    nc.vector.tensor_tensor_scan(gr[:, :w], r2bc[:, hg, :w],
                                                 vr[:, :w], init_r,
                                                 op0=A.mult, op1=A.add)
                    nc.vector.tensor_tensor_scan(gi[:, :w], r2bc[:, hg, :w],
                                                 vi[:, :w], init_i,
                                                 op0=A.mult, op1=A.add)
                    gprev = (gr, gi, w)

                    # output: even t=2j -> C.Gr - S.Gi ; odd -> P1.Gr-P2.Gi+uo
                    o1 = opool.tile([128, W], F16, tag="o1")
                    o2 = opool.tile([128, W], F16, tag="o2")
                    o3 = opool.tile([128, W], F16, tag="o3")
                    o4 = opool.tile([128, W], F16, tag="o4")
                    res_e = work.tile([128, W], F16, tag="res_e")
                    res_o = work.tile([128, W], F16, tag="res_o")
                    nc.vector.tensor_mul(o1[:, :w], cw, gr[:, :w])
                    nc.vector.tensor_mul(o2[:, :w], sw, gi[:, :w])
                    nc.vector.tensor_mul(o3[:, :w], p1t[:, jsl], gr[:, :w])
                    nc.vector.tensor_mul(o4[:, :w], p2t[:, jsl], gi[:, :w])
                    pending.append(dict(pcs=pcs, o1=o1, o2=o2, o3=o3, o4=o4,
                                        res_e=res_e, res_o=res_o,
                                        b=b, hsl=hsl, jsl=jsl, w=w))

        while pending:
            flush_one(pending.pop(0))

    nc.compile()
    return nc


def _prepare(inputs):
    x = np.asarray(inputs["x"], dtype=np.float32)
    B_real = np.asarray(inputs["B_real"], dtype=np.float64)
    B_img = np.asarray(inputs["B_img"], dtype=np.float64)
    nu = np.asarray(inputs["nu"], dtype=np.float64)
    theta = np.asarray(inputs["theta"], dtype=np.float64)
    delta = np.asarray(inputs["delta"], dtype=np.float64)
    h0r = np.asarray(inputs["h0_real"], dtype=np.float64)
    h0i = np.asarray(inputs["h0_img"], dtype=np.float64)

    r = np.exp(-np.exp(nu))
    btr = B_real * delta[None, :]
    bti = B_img * delta[None, :]
    rc = r * np.cos(theta)
    rs = r * np.sin(theta)
    btr2 = btr * rc[None, :] - bti * rs[None, :]
    bti2 = btr * rs[None, :] + bti * rc[None, :]

    jj = np.arange(J, dtype=np.float64)
    ang_c = theta[:, None] * (2.0 * jj + 2.0)[None, :]   # 2theta(j+1)
    ctab = np.cos(ang_c)
    stab = np.sin(ang_c)
    ang_p = theta[:, None] * (2.0 * jj + 3.0)[None, :]   # theta(2j+3)
    p1 = r[:, None] * np.cos(ang_p)
    p2 = r[:, None] * np.sin(ang_p)

    # G_{-1} = h0 / lam = h0 * e^{-i theta} / r
    ginr = (h0r * np.cos(theta) + h0i * np.sin(theta)) / r
    gini = (h0i * np.cos(theta) - h0r * np.sin(theta)) / r

    xh = x.astype(np.float16)
    # host-side transpose to [B, F, J] so device loads are contiguous
    xe = np.ascontiguousarray(xh[:, 0::2, :].transpose(0, 2, 1))   # (B, F, J)
    xo = np.zeros((B, F, JOP), dtype=np.float16)
    xo[:, :, 1:J + 1] = xh[:, 1::2, :].transpose(0, 2, 1)          # slot p = x_{2p-1}

    return dict(
        btr=btr.astype(np.float16), bti=bti.astype(np.float16),
        btr2=btr2.astype(np.float16), bti2=bti2.astype(np.float16),
        r2vec=(r * r).astype(np.float32),
        ctab=ctab.astype(np.float16), stab=stab.astype(np.float16),
        p1tab=p1.astype(np.float16), p2tab=p2.astype(np.float16),
        ginr=ginr.astype(np.float32), gini=gini.astype(np.float32),
    ), xe, xo


_NC_CACHE = {}


def get_program():
    if "nc" not in _NC_CACHE:
        _NC_CACHE["nc"] = build_program()
    return _NC_CACHE["nc"]


def make_in_maps(inputs):
    shared, xe, xo = _prepare(inputs)
    return [dict(xe=np.ascontiguousarray(xe[c * B_LOC:(c + 1) * B_LOC]),
                 xo=np.ascontiguousarray(xo[c * B_LOC:(c + 1) * B_LOC]),
                 **shared)
            for c in range(N_CORES)]


def kernel(**inputs) -> np.ndarray:
    from concourse.bass_utils import run_bass_kernel_spmd

    nc = get_program()
    in_maps = make_in_maps(inputs)
    res = run_bass_kernel_spmd(nc, in_maps, list(range(N_CORES)))
    out = np.empty((B, L, H), dtype=np.float32)
    for c in range(N_CORES):
        sl = slice(c * B_LOC, (c + 1) * B_LOC)
        out[sl, 0::2, :] = res.results[c]["oute"].transpose(0, 2, 1)
        out[sl, 1::2, :] = res.results[c]["outo"].transpose(0, 2, 1)
    return out

